# revision 1
# baseline (speedup 1.0000x reference)
"""AFT full attention (nn_AFTFullAttention) — 8-core TRN2 Bass kernel.

The reference reshapes the contiguous [B, T, H*HD] qkv projections straight
to [B, H, T, HD] (torch .view), so "head" h is a block of T/H = 256 original
time rows per batch, reinterpreted as a [2048, 128] matrix.  Sharding one
head per core therefore gives each core complete channel rows: the batch
reduction (denom / weighted) is head-local AND the output projection is
row-parallel — no collectives at all.

Per core (head h): own rows r_loc = b*256 + t_loc (4 batches x 256 rows).
  P*T = W*.T.T @ x_own.T     -> [c, row] tiles; exp/sigmoid/bias fused into
                                the PSUM evacuation.
  The AFT view [tau, delta] of a [row, c] matrix has partition(delta) =
  c % 128, so Ṽ.T / sigmoid(Q̃).T / Y.T are strided AP views of the
  [c, row] stores; only the exp(K̃) lhsT tiles need PE transposes (64x),
  interleaved just-in-time with the first tau-chunk's numer matmuls.
  numerT_b = ek_b.T.T @ ewT  (lhsT = ek tiles [s,128], rhs = exp(wbias.T))
  outT     = woT.T @ Y.T     row-parallel, + bo fused, wo streamed per dt.
All matmuls in float32r (FP22, full PE rate at N=512).
"""

import os
import sys

sys.path.insert(0, "/opt/trn_rl_repo")

import numpy as np

B, T, DIM, H, HD = 4, 2048, 1024, 8, 128
NCORES = 8
TB = T // H          # 256 original rows per (batch, head-block)
RS = B * TB          # 1024 rows owned per core

KT = DIM // 128      # 8 contraction tiles (dim / c)
ST = T // 128        # 16 s-tiles of the AFT contraction
TC2 = T // 512       # 4 tau-chunks of 512
RC = RS // 512       # 2 row-chunks of 512

TRACE = False        # set by test.py for profiling runs


def _install_ntff_hook():
    """The agent image's antenv lacks axon_hooks; recreate it so
    run_bass_kernel_spmd(trace=True) can capture NTFF profiles."""
    import types

    try:
        from antenv.axon_hooks import get_axon_ntff_profile_hook  # noqa: F401
        return
    except ImportError:
        pass
    import antenv

    mod = types.ModuleType("antenv.axon_hooks")
    _h = [None]
    mod.set_axon_ntff_profile_hook = lambda h: _h.__setitem__(0, h)
    mod.get_axon_ntff_profile_hook = lambda: _h[0]
    sys.modules["antenv.axon_hooks"] = mod
    antenv.axon_hooks = mod
    from trn_agent_boot.trn_boot import _ntff_profile_via_ctypes

    mod.set_axon_ntff_profile_hook(
        _ntff_profile_via_ctypes("/opt/axon/libaxon_pjrt.so")
    )


def _build():
    import concourse.bacc as bacc
    import concourse.tile as tile
    import concourse.mybir as mybir

    f32 = mybir.dt.float32
    f32r = mybir.dt.float32r
    AF = mybir.ActivationFunctionType
    ALU = mybir.AluOpType

    nc = bacc.Bacc("TRN2", debug=False, num_devices=NCORES)

    xT = nc.dram_tensor("xT", [128, KT * RS], mybir.dt.bfloat16, kind="ExternalInput")
    bf16 = mybir.dt.bfloat16
    wqT = nc.dram_tensor("wqT", [128, KT * DIM], bf16, kind="ExternalInput")
    wkT = nc.dram_tensor("wkT", [128, KT * DIM], bf16, kind="ExternalInput")
    wvT = nc.dram_tensor("wvT", [128, KT * DIM], bf16, kind="ExternalInput")
    bq = nc.dram_tensor("bq", [128, KT], f32, kind="ExternalInput")
    bk = nc.dram_tensor("bk", [128, KT], f32, kind="ExternalInput")
    bv = nc.dram_tensor("bv", [128, KT], f32, kind="ExternalInput")
    wbT = nc.dram_tensor("wbT", [T, T], f32, kind="ExternalInput")
    woT = nc.dram_tensor("woT", [128, KT * DIM], f32, kind="ExternalInput")
    bo = nc.dram_tensor("bo", [128, KT], f32, kind="ExternalInput")
    ident = nc.dram_tensor("ident", [128, 128], f32, kind="ExternalInput")
    out = nc.dram_tensor("out", [DIM, RS], f32, kind="ExternalOutput")

    # [c, row] store free-layout: block j (=c//128) at free j*RS + row.
    # AFT view of rows [r0, r0+n): [128(delta), n, 8] with tau = t*8 + j.
    def aft_view(store, r0, n):
        return store.rearrange("p (j r) -> p j r", j=KT)[
            :, :, r0 : r0 + n
        ].transpose([0, 2, 1])

    with tile.TileContext(nc) as tc:
      with (
        tc.tile_pool(name="const", bufs=1) as constp,
        tc.tile_pool(name="pers", bufs=1) as pers,
        tc.tile_pool(name="s12", bufs=1) as s12p,
        tc.tile_pool(name="s2f", bufs=1) as s2f,
      ):
        # persistent stores
        sq_sb = pers.tile([128, KT * RS], f32r, tag="sq")   # sigmoid(q) -> y
        v_tau = pers.tile([128, B * T], f32, tag="v")       # [delta, b*T+tau]
        wsum = pers.tile([128, T], f32, tag="wsum")         # -> weighted
        den = pers.tile([128, T], f32, tag="den")
        ek_sb = s12p.tile([128, KT * RS], f32, tag="ekp")   # exp(k) [c,row]

        # ---------------- stage 1: qkv projections ---------------------
        with (
            tc.tile_pool(name="s1", bufs=1) as s1p,
            tc.tile_pool(name="s1ps", bufs=1, space="PSUM") as ps1,
        ):
            xts = s1p.tile([128, KT * RS], bf16, tag="xts", bufs=1)
            qtr = KT * RS // 4
            nc.sync.dma_start(out=xts[:, 0:qtr], in_=xT[:, 0:qtr])
            nc.scalar.dma_start(out=xts[:, qtr : 2 * qtr],
                                in_=xT[:, qtr : 2 * qtr])
            nc.sync.dma_start(out=xts[:, 2 * qtr : 3 * qtr],
                              in_=xT[:, 2 * qtr : 3 * qtr])
            nc.scalar.dma_start(out=xts[:, 3 * qtr :], in_=xT[:, 3 * qtr :])
            id_sb = constp.tile([128, 128], f32, tag="id")
            nc.scalar.dma_start(out=id_sb, in_=ident[:])
            bias_sb = {}
            for nm, tsr in [("bq", bq), ("bk", bk), ("bv", bv), ("bo", bo)]:
                t_ = constp.tile([128, KT], f32, tag=nm, name=f"b_{nm}")
                nc.scalar.dma_start(out=t_, in_=tsr[:])
                bias_sb[nm] = t_

            specs = [
                ("q", wqT, AF.Sigmoid, "bq", sq_sb),
                ("k", wkT, AF.Exp, "bk", ek_sb),
                ("v", wvT, AF.Identity, "bv", None),
            ]
            v4 = v_tau.rearrange("p (b t j) -> p b t j", b=B, j=8)
            for j in range(KT):
                for nm, wt, func, bnm, store in specs:
                    wtile = s1p.tile([128, KT * 128], bf16, tag="wt",
                                     bufs=6, name=f"wt_{nm}_{j}")
                    nc.sync.dma_start(
                        out=wtile,
                        in_=wt[:, j * KT * 128 : (j + 1) * KT * 128],
                    )
                    for rc in range(RC):
                        psum = ps1.tile([128, 512], f32, tag="qkv", bufs=4,
                                        name=f"ps_{nm}_{j}_{rc}")
                        for kt in range(KT):
                            nc.tensor.matmul(
                                psum,
                                wtile[:, kt * 128 : (kt + 1) * 128],
                                xts[:, rc * (KT * 512) + kt * 512 :
                                    rc * (KT * 512) + (kt + 1) * 512],
                                start=(kt == 0),
                                stop=(kt == KT - 1),
                            )
                        if store is None:
                            nc.vector.tensor_scalar_add(
                                out=v4[:, rc * 2 : (rc + 1) * 2, :, j],
                                in0=psum.rearrange("p (b t) -> p b t", b=2),
                                scalar1=bias_sb[bnm][:, j : j + 1],
                            )
                        else:
                            nc.scalar.activation(
                                out=store[:, j * RS + rc * 512 :
                                          j * RS + (rc + 1) * 512],
                                in_=psum, func=func,
                                bias=bias_sb[bnm][:, j : j + 1],
                            )

        # ---------------- stages 2+3 ----------------------------------
        with tc.tile_pool(name="s3", bufs=1) as s3p:
            with (
                tc.tile_pool(name="s2", bufs=1) as s2p,
                tc.tile_pool(name="s2ps", bufs=1, space="PSUM") as ps2,
            ):
                eks_sb = s2p.tile([128, B * T], bf16, tag="eks", bufs=1)
                for tc2 in range(TC2):
                    tsl = slice(tc2 * 512, (tc2 + 1) * 512)
                    nps = [ps2.tile([128, 512], f32, tag="np", bufs=5,
                                    name=f"np_{tc2}_{b}") for b in range(B)]
                    for st in range(ST):
                        raw = s2f.tile([128, 512], f32, tag="raw", bufs=6,
                                       name=f"raw_{tc2}_{st}")
                        nc.gpsimd.dma_start(
                            out=raw, in_=wbT[st * 128 : (st + 1) * 128, tsl]
                        )
                        ewt = s2f.tile([128, 512], bf16, tag="ew", bufs=6,
                                       name=f"ew_{tc2}_{st}")
                        nc.scalar.activation(out=ewt, in_=raw, func=AF.Exp)
                        if tc2 == 0:
                            # build the ek lhsT tiles for this st in time
                            for b in range(B):
                                view = aft_view(ek_sb, b * TB + st * 16, 16)
                                dvt = s2f.tile([128, 128], f32, tag="dvt",
                                               bufs=6, name=f"dvt_{b}_{st}")
                                nc.vector.tensor_copy(
                                    out=dvt.rearrange(
                                        "p (a c) -> p a c", c=8),
                                    in_=view,
                                )
                                tp = ps2.tile([128, 128], f32, tag="tr",
                                              bufs=3, name=f"tp_{b}_{st}")
                                nc.tensor.transpose(tp, dvt, id_sb)
                                blk = b * ST + st
                                nc.vector.tensor_copy(
                                    out=eks_sb[:, blk * 128 :
                                               (blk + 1) * 128],
                                    in_=tp,
                                )
                        for b in range(B):
                            blk = b * ST + st
                            nc.tensor.matmul(
                                nps[b],
                                eks_sb[:, blk * 128 : (blk + 1) * 128],
                                ewt,
                                start=(st == 0),
                                stop=(st == ST - 1),
                            )
                    for b in range(B):
                        vview = v_tau[:, b * T + tc2 * 512 :
                                      b * T + (tc2 + 1) * 512]
                        if b == 0:
                            nc.vector.tensor_tensor(
                                out=wsum[:, tsl], in0=nps[b], in1=vview,
                                op=ALU.mult,
                            )
                            nc.vector.tensor_copy(out=den[:, tsl],
                                                  in_=nps[b])
                        else:
                            nv = s2f.tile([128, 512], f32, tag="nv", bufs=3,
                                          name=f"nv_{tc2}_{b}")
                            nc.vector.tensor_tensor(
                                out=nv, in0=nps[b], in1=vview, op=ALU.mult,
                            )
                            nc.vector.tensor_add(
                                out=wsum[:, tsl], in0=wsum[:, tsl], in1=nv
                            )
                            nc.vector.tensor_add(
                                out=den[:, tsl], in0=den[:, tsl], in1=nps[b]
                            )
                    # incremental weighted + y for this tau-chunk
                    rec = s2f.tile([128, 512], f32, tag="rec", bufs=2,
                                   name=f"rec_{tc2}")
                    nc.vector.reciprocal_approx_fast(out=rec,
                                                     in_=den[:, tsl])
                    nc.vector.tensor_tensor(out=wsum[:, tsl],
                                            in0=wsum[:, tsl],
                                            in1=rec, op=ALU.mult)
                    for b in range(B):
                        sqv = aft_view(sq_sb, b * TB + tc2 * 64, 64)
                        wgv = wsum[:, tsl].rearrange("p (a c) -> p a c", c=8)
                        nc.vector.tensor_tensor(
                            out=sqv, in0=sqv, in1=wgv, op=ALU.mult,
                        )

            # ------------ stage 3: out projection, wo streamed --------
            with tc.tile_pool(name="s3ps", bufs=1, space="PSUM") as ps3:
                for dt_ in range(KT):
                    wod = s3p.tile([128, KT * 128], f32r, tag="wod",
                                   bufs=3, name=f"wod_{dt_}")
                    nc.sync.dma_start(
                        out=wod,
                        in_=woT[:, dt_ * KT * 128 :
                                (dt_ + 1) * KT * 128].bitcast(f32r),
                    )
                    for rc in range(RC):
                        rsl = slice(rc * 512, (rc + 1) * 512)
                        pso = ps3.tile([128, 512], f32, tag="o", bufs=4,
                                       name=f"pso_{rc}_{dt_}")
                        for j in range(KT):
                            nc.tensor.matmul(
                                pso,
                                wod[:, j * 128 : (j + 1) * 128],
                                sq_sb[:, j * RS + rc * 512 :
                                      j * RS + (rc + 1) * 512],
                                start=(j == 0),
                                stop=(j == KT - 1),
                            )
                        osb = s3p.tile([128, 512], f32, tag="ot", bufs=3,
                                       name=f"osb_{rc}_{dt_}")
                        nc.scalar.activation(
                            out=osb, in_=pso, func=AF.Identity,
                            bias=bias_sb["bo"][:, dt_ : dt_ + 1],
                        )
                        nc.sync.dma_start(
                            out=out[dt_ * 128 : (dt_ + 1) * 128, rsl],
                            in_=osb,
                        )

    nc.compile()
    return nc


_NC_CACHE = None


def kernel(x, Wq, bq, Wk, bk, Wv, bv, wbias, Wo, bo):
    global _NC_CACHE
    from concourse import bass_utils

    f = np.float32
    x = np.asarray(x, f)
    Wq, Wk, Wv, Wo = (np.asarray(a, f) for a in (Wq, Wk, Wv, Wo))
    bq, bk, bv, bo = (np.asarray(a, f) for a in (bq, bk, bv, bo))
    wbias = np.asarray(wbias, f)

    x2 = x.reshape(B * T, DIM)

    def tile_w(W):
        # host[p, X*1024 + Y*128 + d] = W[X*128+d, Y*128+p]
        return np.ascontiguousarray(
            W.reshape(KT, 128, KT, 128).transpose(3, 0, 2, 1).reshape(
                128, KT * KT * 128)
        )

    # qkv: X = out-channel block j, Y = contraction block kt  -> need
    #   host[p, j*1024 + kt*128 + d] = W[j*128+d, kt*128+p]      = tile_w(W)
    import ml_dtypes
    bf = ml_dtypes.bfloat16
    wqT = tile_w(Wq).astype(bf)
    wkT = tile_w(Wk).astype(bf)
    wvT = tile_w(Wv).astype(bf)
    # wo: X = out-dim block dt, Y = contraction block j         = tile_w(Wo)
    woT = tile_w(Wo)
    id_np = np.eye(128, dtype=f)
    bqc = np.ascontiguousarray(bq.reshape(KT, 128).T)
    bkc = np.ascontiguousarray(bk.reshape(KT, 128).T)
    bvc = np.ascontiguousarray(bv.reshape(KT, 128).T)
    boc = np.ascontiguousarray(bo.reshape(KT, 128).T)

    in_maps = []
    for c in range(NCORES):
        rows = np.concatenate(
            [x2[b * T + c * TB : b * T + (c + 1) * TB] for b in range(B)]
        )  # [RS, DIM], row = b*TB + t_loc
        in_maps.append({
            "xT": np.ascontiguousarray(
                rows.T.reshape(KT, 128, RC, 512).transpose(1, 2, 0, 3)
                .reshape(128, KT * RS)).astype(bf),
            "wqT": wqT, "wkT": wkT, "wvT": wvT,
            "bq": bqc, "bk": bkc, "bv": bvc,
            "wbT": np.ascontiguousarray(wbias[c].T),
            "woT": woT, "bo": boc, "ident": id_np,
        })

    if TRACE:
        _install_ntff_hook()
    if _NC_CACHE is None:
        _NC_CACHE = _build()
    nc = _NC_CACHE

    res = bass_utils.run_bass_kernel_spmd(
        nc, in_maps, core_ids=list(range(NCORES)), trace=TRACE
    )
    outf = np.empty((B * T, DIM), f)
    for c in range(NCORES):
        blk = res.results[c]["out"].T  # [RS, DIM], row = b*TB + t_loc
        for b in range(B):
            outf[b * T + c * TB : b * T + (c + 1) * TB] = (
                blk[b * TB : (b + 1) * TB]
            )
    if TRACE:
        kernel.last_exec_time_ns = res.exec_time_ns
        kernel.last_results = res
    return outf.reshape(B, T, DIM)

